# revision 4
# baseline (speedup 1.0000x reference)
"""Trainium2 Bass kernel v2 for CustomGATLayerIsotropic (GNN message passing).

Math:
    z   = einsum('nd,hod->nho', h, W);  agg = segment_sum(z[src], dst, N)
    out = h + elu(BN(agg))
Identity: project AFTER aggregating (segment_sum is linear):
    agg = aggh @ Wt  where  aggh = segment_sum(h[src], dst, N)

v2 design (per core, dst-sharded nodes npc=N/8):
  - per-edge gather of h rows (bf16, 512B) via SWDGE dma_gather with the
    4 SWDGE queues round-robined (Q7 desc-gen parallelizes ~3.5x)
  - dst windows of 256 nodes; segment-sum via one-hot matmul in TRANSPOSED
    orientation:  segT[dh][d, dst] += buf(tile)[slot, d-half]^T @ sel[slot, dst]
    -> aggh^T produced directly; no per-window transposes, no gram matrices
  - sel generated per (window, src-window) group in one batched DVE
    tensor_tensor (iota == ldst) using broadcast access patterns
  - per-window projection uT = W^T aggh^T one window behind; BN stats
    (sum, sum^2 per channel) via Act accum_out while copying u to SBUF bf16
  - tiny [128,4] f32 stats AllReduce; finish = scale/bias + ELU + residual
    on [128,npc] strips with hoT loaded per chunk during the tail.
"""

import math
import sys

sys.path.insert(0, "/opt/trn_rl_repo")

import numpy as np
import ml_dtypes

import concourse.bass as bass
import concourse.tile as tile
from concourse import bacc, mybir
from concourse import bass_utils

F32 = mybir.dt.float32
BF16 = mybir.dt.bfloat16
I16 = mybir.dt.int16

FULL_CFG = dict(
    N=100000,
    E=1600000,
    D=256,
    H=8,
    O=32,
    EPS=1e-5,
    n_cores=8,
    src_win=32768,   # int16 gather index limit
    chunk=1024,      # rows per dma_gather call (= SWDGE ring capacity)
    n_queues=4,      # SWDGE queues round-robined for desc-gen parallelism
    WD=128,          # dst-window width (nodes per PSUM accumulation group)
)


# --------------------------------------------------------------------------
# Host-side preprocessing: edge partitioning / sorting / index construction
# --------------------------------------------------------------------------

def preprocess(cfg, src, dst):
    """Static schedule + per-core gather indices / local-dst tiles.

    Edges of core c (dst in its shard) sorted by (dst window w, src window k);
    each (w,k) group padded to whole 128-row tiles; per-k gather streams with
    int16 indices; ldst holds dst&(WD-1) per slot (-1 pad)."""
    N, E = cfg["N"], cfg["E"]
    n_cores = cfg["n_cores"]
    src_win = cfg["src_win"]
    chunk = cfg["chunk"]
    WD = cfg["WD"]
    npc = N // n_cores
    nw = (npc + WD - 1) // WD
    nk = (N + src_win - 1) // src_win
    wshift = WD.bit_length() - 1

    cnt = np.zeros((n_cores, nw, nk), np.int64)
    per_core_raw = []
    for c in range(n_cores):
        base = c * npc
        sel = (dst >= base) & (dst < base + npc)
        s = src[sel].astype(np.int64)
        d = dst[sel].astype(np.int64) - base
        w = d >> wshift
        k = s // src_win
        order = np.lexsort((k, w))
        s, d, w, k = s[order], d[order], w[order], k[order]
        cnt[c] = np.bincount(w * nk + k, minlength=nw * nk).reshape(nw, nk)
        per_core_raw.append((s, d, w, k))

    g = cnt.max(axis=0)
    empty_w = g.sum(axis=1) == 0
    g[empty_w, 0] = 1

    # stream layout: stream k = concat over w of g[w,k] slots (unaligned!)
    seg_off = np.zeros((nw, nk), np.int64)
    L = np.zeros(nk, np.int64)
    for k in range(nk):
        off = 0
        for w in range(nw):
            seg_off[w, k] = off
            off += g[w, k]
        L[k] = off
    Lp = ((L + chunk - 1) // chunk) * chunk
    n_calls = Lp // chunk

    # runs: (w, k, tile) with tile spanning the group's slot range
    runs = []            # flat list in consumption order (w, k, t)
    run_col = {}
    for w in range(nw):
        for k in range(nk):
            if g[w, k] == 0:
                continue
            t0 = int(seg_off[w, k] // 128)
            t1 = int((seg_off[w, k] + g[w, k] - 1) // 128)
            for t in range(t0, t1 + 1):
                run_col[(w, k, t)] = len(runs)
                runs.append((w, k, t))
    n_runs = len(runs)
    span = np.zeros((nw, nk), np.int64)
    for w in range(nw):
        for k in range(nk):
            if g[w, k] > 0:
                span[w, k] = ((seg_off[w, k] + g[w, k] - 1) // 128
                              - seg_off[w, k] // 128 + 1)

    valid = []
    for k in range(nk):
        v = []
        for ci in range(int(n_calls[k])):
            v.append(int(min(chunk, max(0, L[k] - ci * chunk))))
        valid.append(v)

    rows_k = [min(src_win, N - k * src_win) for k in range(nk)]

    sched = dict(
        npc=npc, nw=nw, nk=nk, g=g, L=L, Lp=Lp, n_calls=n_calls,
        seg_off=seg_off, runs=runs, run_col=run_col, n_runs=n_runs,
        span=span, tpc=chunk // 128, rows_k=rows_k, valid=valid,
        Tkmax=int(span.max()), WD=WD,
    )

    per_core = []
    for c in range(n_cores):
        s, d, w, k = per_core_raw[c]
        gid = w * nk + k
        starts = np.zeros(nw * nk + 1, np.int64)
        np.cumsum(np.bincount(gid, minlength=nw * nk), out=starts[1:])
        rank = np.arange(len(s)) - starts[gid]
        slot = seg_off[w, k] + rank
        idxs = []
        for kk in range(nk):
            arr = np.zeros(int(Lp[kk]), np.int64)
            m = k == kk
            arr[slot[m]] = s[m] - kk * src_win
            arr[int(L[kk]):] = -1
            a16 = arr.astype(np.int16)
            wrapped = a16.reshape(-1, 16).T
            idxs.append(np.tile(wrapped, (8, 1)))
        # ldst per RUN column: slot position within the run's tile
        ldst = np.full((128, n_runs), -1.0, np.float32)
        tile_of_slot = slot // 128
        cols = np.array([run_col[(int(wi), int(ki), int(ti))]
                         for wi, ki, ti in zip(w, k, tile_of_slot)])
        ldst[slot & 127, cols] = (d & (WD - 1)).astype(np.float32)
        per_core.append(dict(idxs=idxs, ldst=ldst))

    return sched, per_core


# --------------------------------------------------------------------------
# Bass program builder (SPMD; identical across cores)
# --------------------------------------------------------------------------

def build_nc(cfg, sched, no_collective=False):
    N = cfg["N"]
    D = cfg["D"]
    C = cfg["H"] * cfg["O"]
    n_cores = cfg["n_cores"]
    nq = cfg["n_queues"]
    WD = cfg["WD"]
    npc, nw, nk = sched["npc"], sched["nw"], sched["nk"]
    g, tpc = sched["g"], sched["tpc"]
    seg_off, span = sched["seg_off"], sched["span"]
    n_runs, Tkmax = sched["n_runs"], sched["Tkmax"]
    n_calls, valid, rows_k = sched["n_calls"], sched["valid"], sched["rows_k"]
    Lp = sched["Lp"]
    AL = mybir.AluOpType
    AF = mybir.ActivationFunctionType

    nc = bacc.Bacc("TRN2", target_bir_lowering=False, debug=False,
                   num_devices=n_cores,
                   dynamic_dma_scratch_size=16384,
                   num_swdge_queues=nq)

    hm_d = nc.dram_tensor("hm", [N, D], BF16, kind="ExternalInput")
    hoT_d = nc.dram_tensor("hoT", [128, 2 * npc], BF16, kind="ExternalInput")
    wt_d = nc.dram_tensor("wt", [D, C], BF16, kind="ExternalInput")
    gam_d = nc.dram_tensor("gamT", [128, 2], F32, kind="ExternalInput")
    bet_d = nc.dram_tensor("betT", [128, 2], F32, kind="ExternalInput")
    iota_d = nc.dram_tensor("iota", [128, sched["Tkmax"] * WD], BF16,
                            kind="ExternalInput")
    ldst_d = nc.dram_tensor("ldst", [128, n_runs], BF16, kind="ExternalInput")
    idx_d = [nc.dram_tensor(f"idx{k}", [128, int(Lp[k]) // 16], I16,
                            kind="ExternalInput") for k in range(nk)]
    out_d = nc.dram_tensor("out", [128, 2 * npc], BF16, kind="ExternalOutput")
    stats_in_d = nc.dram_tensor("stats_in", [128, 4], F32)
    stats_out_d = nc.dram_tensor("stats_out", [128, 4], F32)

    from contextlib import ExitStack

    with tile.TileContext(nc) as tc, ExitStack() as ctx:
        singles = ctx.enter_context(tc.tile_pool(name="singles", bufs=1))
        persist = ctx.enter_context(tc.tile_pool(name="persist", bufs=1))

        # ---- static tiles -------------------------------------------------
        iota_sb = singles.tile([128, Tkmax * WD], BF16)
        nc.sync.dma_start(out=iota_sb[:], in_=iota_d[:, :])
        ldst_sb = singles.tile([128, n_runs], BF16)
        nc.sync.dma_start(out=ldst_sb[:], in_=ldst_d[:, :])
        idx_sb = []
        for k in range(nk):
            t_ = singles.tile([128, int(Lp[k]) // 16], I16, name=f"idx_sb{k}")
            nc.sync.dma_start(out=t_[:], in_=idx_d[k][:, :])
            idx_sb.append(t_)
        wt_sb = []
        for j in range(2):
            t_ = singles.tile([128, C], BF16, name=f"wt_sb{j}")
            nc.sync.dma_start(out=t_[:], in_=wt_d[j * 128:(j + 1) * 128, :])
            wt_sb.append(t_)
        gam_sb = singles.tile([128, 2], F32)
        nc.sync.dma_start(out=gam_sb[:], in_=gam_d[:, :])
        bet_sb = singles.tile([128, 2], F32)
        nc.sync.dma_start(out=bet_sb[:], in_=bet_d[:, :])
        eps_t = singles.tile([128, 1], F32)
        nc.vector.memset(eps_t[:], cfg["EPS"])

        # persistent state
        u_sb = persist.tile([128, 2, nw * WD], BF16, name="u_sb")
        sum_strip = persist.tile([128, 2, nw // 2], F32, name="sum_strip")
        sumsq_strip = persist.tile([128, 2, nw // 2], F32, name="sumsq_strip")
        sq_scratch = persist.tile([128, 2 * WD], BF16, name="sq_scratch")

        with ExitStack() as p1:
            aggp = p1.enter_context(tc.tile_pool(name="aggp", bufs=1))
            agghT = [aggp.tile([128, nw * WD], BF16, name=f"agghT{j}")
                     for j in range(2)]
            stream_pools = [
                p1.enter_context(tc.tile_pool(name=f"gbuf{k}", bufs=3))
                for k in range(nk)
            ]
            selp = p1.enter_context(tc.tile_pool(name="selp", bufs=3))
            seg_ps = p1.enter_context(
                tc.tile_pool(name="segps", bufs=2, space="PSUM"))
            u_ps_pool = p1.enter_context(
                tc.tile_pool(name="ups", bufs=2, space="PSUM"))

            def project_pair(w):
                """uT(w..w+1) from agghT; copy to u_sb + stats via Act accum."""
                for ch in range(2):
                    u_ps = u_ps_pool.tile([128, 2 * WD], F32, name="u_ps")
                    for dh in range(2):
                        nc.tensor.matmul(
                            u_ps[:],
                            lhsT=wt_sb[dh][:, ch * 128:(ch + 1) * 128],
                            rhs=agghT[dh][:, w * WD:(w + 2) * WD],
                            start=(dh == 0), stop=(dh == 1))
                    nc.scalar.activation(
                        out=u_sb[:, ch, w * WD:(w + 2) * WD], in_=u_ps[:],
                        func=AF.Identity,
                        accum_out=sum_strip[:, ch, w // 2:w // 2 + 1])
                    nc.scalar.activation(
                        out=sq_scratch[:], in_=u_ps[:],
                        func=AF.Square,
                        accum_out=sumsq_strip[:, ch, w // 2:w // 2 + 1])

            chunk_buf = [dict() for _ in range(nk)]   # ci -> buffer
            next_chunk = [0] * nk
            qrr = 0
            col = 0
            for w in range(nw):
                tw = int(span[w].sum())          # runs in this window
                seg = [seg_ps.tile([128, WD], F32, name=f"seg{j}")
                       for j in range(2)]
                ti = 0
                for k in range(nk):
                    if g[w, k] == 0:
                        continue
                    t0 = int(seg_off[w, k] // 128)
                    t1 = int((seg_off[w, k] + g[w, k] - 1) // 128)
                    # gather chunks covering tiles t0..t1
                    while next_chunk[k] * tpc <= t1:
                        ci = next_chunk[k]
                        cpc = cfg["chunk"] // 16
                        buf = stream_pools[k].tile(
                            [128, tpc, D], BF16, name=f"gbuf{k}")
                        nc.gpsimd.dma_gather(
                            buf[:],
                            hm_d[k * cfg["src_win"]:
                                 k * cfg["src_win"] + rows_k[k], :],
                            idx_sb[k][:, ci * cpc:(ci + 1) * cpc],
                            cfg["chunk"],
                            valid[k][ci],
                            D,
                            queue_num=qrr,
                        )
                        qrr = (qrr + 1) % nq
                        chunk_buf[k][ci] = buf
                        if ci >= 3:
                            chunk_buf[k].pop(ci - 3, None)
                        next_chunk[k] = ci + 1
                    spn = t1 - t0 + 1
                    sel = selp.tile([128, Tkmax, WD], BF16, name="sel")
                    lb = ldst_sb[:, col:col + spn]
                    ap_ldst = bass.AP(
                        tensor=lb.tensor, offset=lb.offset,
                        ap=[lb.ap[0], [1, spn], [0, WD]])
                    ib = iota_sb[:, 0:spn * WD]
                    ap_iota = bass.AP(
                        tensor=ib.tensor, offset=ib.offset,
                        ap=[ib.ap[0], [WD, spn], [1, WD]])
                    nc.vector.tensor_tensor(
                        out=sel[:, :spn, :], in0=ap_iota,
                        in1=ap_ldst, op=AL.is_equal)
                    for rj, t in enumerate(range(t0, t1 + 1)):
                        buf = chunk_buf[k][t // tpc]
                        sl = t % tpc
                        for dh in range(2):
                            nc.tensor.matmul(
                                seg[dh][:],
                                lhsT=buf[:, sl, dh * 128:(dh + 1) * 128],
                                rhs=sel[:, rj, :],
                                start=(ti == 0), stop=(ti == tw - 1))
                        ti += 1
                    col += spn
                for dh in range(2):
                    nc.scalar.copy(
                        out=agghT[dh][:, w * WD:(w + 1) * WD],
                        in_=seg[dh][:])
                if w >= 2 and w % 2 == 0:
                    project_pair(w - 2)
            project_pair(nw - 2)

        # ---- stats + allreduce -------------------------------------------
        with ExitStack() as p15:
            sp = p15.enter_context(tc.tile_pool(name="statp", bufs=1))
            stats_sb = sp.tile([128, 4], F32)
            nc.vector.tensor_reduce(
                out=stats_sb[:, 0:2], in_=sum_strip[:],
                axis=mybir.AxisListType.X, op=AL.add)
            nc.vector.tensor_reduce(
                out=stats_sb[:, 2:4], in_=sumsq_strip[:],
                axis=mybir.AxisListType.X, op=AL.add)
            nc.sync.dma_start(out=stats_in_d[:, :], in_=stats_sb[:])
            if no_collective:
                nc.sync.dma_start(out=stats_out_d[:, :], in_=stats_in_d[:, :])
            else:
                nc.gpsimd.collective_compute(
                    "AllReduce", AL.add,
                    replica_groups=[list(range(n_cores))],
                    ins=[stats_in_d.ap().opt()],
                    outs=[stats_out_d.ap().opt()],
                )
            stats_g = sp.tile([128, 4], F32)
            nc.sync.dma_start(out=stats_g[:], in_=stats_out_d[:, :])

            mv = sp.tile([128, 4], F32)
            nc.vector.tensor_scalar_mul(mv[:], stats_g[:, :], 1.0 / N)
            mean = mv[:, 0:2]
            var = mv[:, 2:4]
            m2 = sp.tile([128, 2], F32)
            nc.vector.tensor_mul(m2[:], mean, mean)
            nc.vector.tensor_sub(var, var, m2[:])
            sd = sp.tile([128, 2], F32)
            nc.scalar.activation(out=sd[:], in_=var, func=AF.Sqrt,
                                 bias=eps_t[:, 0:1])
            rstd = sp.tile([128, 2], F32)
            nc.vector.reciprocal(out=rstd[:], in_=sd[:])
            scale_p = persist.tile([128, 2], F32, name="scale_p")
            nc.vector.tensor_mul(scale_p[:], rstd[:], gam_sb[:])
            bias_p = persist.tile([128, 2], F32, name="bias_p")
            tmp = sp.tile([128, 2], F32)
            nc.vector.tensor_mul(tmp[:], mean, scale_p[:])
            nc.vector.tensor_sub(bias_p[:], bet_sb[:], tmp[:])

            # ---- finish: scale/bias + ELU + residual on strips -----------
            CHN = 8
            clen = (npc + CHN - 1) // CHN
            fpool = p15.enter_context(tc.tile_pool(name="fin", bufs=3))
            hop = p15.enter_context(tc.tile_pool(name="hop", bufs=2))
            for ch in range(2):
                for q in range(CHN):
                    lo = q * clen
                    hi = min(lo + clen, npc)
                    ln = hi - lo
                    ho = hop.tile([128, clen], BF16, name="ho")
                    nc.sync.dma_start(
                        out=ho[:, :ln],
                        in_=hoT_d[:, ch * npc + lo:ch * npc + hi])
                    tt = fpool.tile([128, clen], BF16, name="tt")
                    nc.vector.tensor_scalar(
                        out=tt[:, :ln], in0=u_sb[:, ch, lo:hi],
                        scalar1=scale_p[:, ch:ch + 1],
                        scalar2=bias_p[:, ch:ch + 1],
                        op0=AL.mult, op1=AL.add)
                    ee = fpool.tile([128, clen], BF16, name="ee")
                    ss = fpool.tile([128, clen], BF16, name="ss")
                    nc.scalar.activation(out=ee[:, :ln], in_=tt[:, :ln],
                                         func=AF.Exp)
                    nc.vector.scalar_tensor_tensor(
                        out=ss[:, :ln], in0=tt[:, :ln], scalar=0.0,
                        in1=ho[:, :ln], op0=AL.max, op1=AL.add)
                    oo = fpool.tile([128, clen], BF16, name="oo")
                    nc.vector.scalar_tensor_tensor(
                        out=oo[:, :ln], in0=ee[:, :ln], scalar=1.0,
                        in1=ss[:, :ln], op0=AL.min, op1=AL.add)
                    nc.sync.dma_start(
                        out=out_d[:, ch * npc + lo:ch * npc + hi],
                        in_=oo[:, :ln])

    nc.compile()
    return nc


# --------------------------------------------------------------------------
# Host orchestration
# --------------------------------------------------------------------------

def make_in_maps(cfg, sched, per_core, h, W, gamma, beta):
    N, D = cfg["N"], cfg["D"]
    C = cfg["H"] * cfg["O"]
    WD = cfg["WD"]
    npc = sched["npc"]
    hm = np.ascontiguousarray(h.astype(ml_dtypes.bfloat16))
    wt = np.ascontiguousarray(
        np.transpose(W, (2, 0, 1)).reshape(D, C).astype(ml_dtypes.bfloat16))
    gamT = np.ascontiguousarray(
        gamma.reshape(C).astype(np.float32).reshape(2, 128).T)
    betT = np.ascontiguousarray(
        beta.reshape(C).astype(np.float32).reshape(2, 128).T)
    iota = np.ascontiguousarray(
        np.tile(np.arange(WD, dtype=np.float32), (128, sched["Tkmax"]))
        .astype(ml_dtypes.bfloat16))
    in_maps = []
    for c in range(cfg["n_cores"]):
        pc = per_core[c]
        hoT = ((h[c * npc:(c + 1) * npc].astype(np.float32) - 1.0).T
               .reshape(2, 128, npc).transpose(1, 0, 2).reshape(128, 2 * npc))
        m = {
            "hm": hm,
            "hoT": np.ascontiguousarray(hoT.astype(ml_dtypes.bfloat16)),
            "wt": wt,
            "gamT": gamT,
            "betT": betT,
            "iota": iota,
            "ldst": np.ascontiguousarray(
                pc["ldst"].astype(ml_dtypes.bfloat16)),
        }
        for k in range(sched["nk"]):
            m[f"idx{k}"] = np.ascontiguousarray(pc["idxs"][k])
        in_maps.append(m)
    return in_maps


def postprocess(cfg, sched, results):
    npc = sched["npc"]
    C = cfg["H"] * cfg["O"]
    out = np.empty((cfg["N"], C), np.float32)
    for c in range(cfg["n_cores"]):
        o = np.asarray(results[c]["out"]).reshape(128, 2, npc)
        out[c * npc:(c + 1) * npc] = (
            o.transpose(1, 0, 2).reshape(C, npc).T.astype(np.float32))
    return out


_CACHE = {}


def kernel(h, W, gamma, beta, src, dst, e):
    cfg = FULL_CFG
    h = np.asarray(h)
    W = np.asarray(W)
    gamma = np.asarray(gamma)
    beta = np.asarray(beta)
    src = np.asarray(src)
    dst = np.asarray(dst)

    sched, per_core = preprocess(cfg, src, dst)
    nc_key = ("nc", tuple(sched["g"].flatten().tolist()))
    if nc_key in _CACHE:
        nc = _CACHE[nc_key]
    else:
        nc = build_nc(cfg, sched)
        _CACHE.clear()
        _CACHE[nc_key] = nc

    in_maps = make_in_maps(cfg, sched, per_core, h, W, gamma, beta)
    res = bass_utils.run_bass_kernel_spmd(
        nc, in_maps, core_ids=list(range(cfg["n_cores"])))
    return postprocess(cfg, sched, res.results)
